# revision 1
# baseline (speedup 1.0000x reference)
"""ARX forward kernel for Trainium2 (8 NeuronCores, data-parallel).

The reference zeroes the exogenous term, so the model is a pure linear
recurrence out[:, t] = sum_k w_k * out[:, t-8+k] with out[:, :8] = y.
Writing the 8x8 companion matrix M (carry_{t+1} = carry_t @ M) gives
pred_t = y @ (M^t w), so the whole 4096-step scan collapses into one
matmul out = y @ [I_8 | V] with V[:, t] = M^t w precomputed on host
(4096 tiny 8-vector iterations, float64).

The recurrence is stable (spectral radius ~0.77 for the 0.05-scaled
weights), so M^t w underflows float32 to exactly 0 after a few hundred
steps; both the reference scan and this kernel produce exact zeros
there.  The device therefore computes and writes only the nonzero
column prefix (determined from V at runtime) and the host pads the
remaining all-zero columns.

Sharding: pure data parallel, batch 8192 -> 1024 rows per core, W/V
replicated, per-core output gathered on host by concatenation.
"""

import numpy as np

AR = 8
SEQ = 4096
BATCH = 8192
OUT_COLS = SEQ + AR          # 4104
N_CORES = 8
ROWS = BATCH // N_CORES      # 1024
P = 128                      # SBUF/PSUM partitions
MM_CHUNK = 512               # max fp32 matmul free dim / one PSUM bank

_nc_cache = {}
LAST_RESULTS = None          # BassKernelResults of the most recent run


def _build_nc(n_cols):
    """Bass/Tile program: out[1024, n_cols] = yT.T @ V  (per core)."""
    import concourse.mybir as mybir
    import concourse.tile as tile
    from concourse import bacc

    f32 = mybir.dt.float32
    nc = bacc.Bacc("TRN2", target_bir_lowering=False, debug=False,
                   num_devices=N_CORES)
    yT = nc.dram_tensor("yT", [AR, ROWS], f32, kind="ExternalInput").ap()
    V = nc.dram_tensor("V", [AR, n_cols], f32, kind="ExternalInput").ap()
    out = nc.dram_tensor("out", [ROWS, n_cols], f32,
                         kind="ExternalOutput").ap()

    chunks = [(c, min(MM_CHUNK, n_cols - c)) for c in range(0, n_cols, MM_CHUNK)]

    with tile.TileContext(nc) as tc:
        with (
            tc.tile_pool(name="const", bufs=1) as cpool,
            tc.tile_pool(name="outs", bufs=3) as opool,
            tc.tile_pool(name="psum", bufs=8, space="PSUM") as ppool,
        ):
            yT_t = cpool.tile([AR, ROWS], f32)
            nc.sync.dma_start(yT_t[:], yT)
            V_t = cpool.tile([AR, n_cols], f32)
            nc.sync.dma_start(V_t[:], V)
            for rc in range(ROWS // P):
                ot = opool.tile([P, n_cols], f32, tag="ot")
                for c, wd in chunks:
                    ps = ppool.tile([P, MM_CHUNK], f32, tag="ps")
                    nc.tensor.matmul(
                        ps[:, :wd],
                        yT_t[:, rc * P:(rc + 1) * P],
                        V_t[:, c:c + wd],
                        start=True, stop=True,
                    )
                    nc.vector.tensor_copy(ot[:, c:c + wd], ps[:, :wd])
                nc.sync.dma_start(out[rc * P:(rc + 1) * P, :], ot[:])
    nc.compile()
    return nc


def _v_table(W):
    """V[:, t] = M^t w in float64, cast to float32.  v_{t+1}[0] = w0*v[7],
    v_{t+1}[i] = v[i-1] + w_i*v[7]."""
    w = np.asarray(W, dtype=np.float64)[0, :AR]
    V = np.zeros((AR, SEQ), dtype=np.float64)
    v = w.copy()
    for t in range(SEQ):
        V[:, t] = v
        nv = np.empty(AR)
        nv[0] = 0.0
        nv[1:] = v[:-1]
        nv += w * v[AR - 1]
        v = nv
        if not np.isfinite(v).all():
            # unstable recurrence: match f32 overflow behaviour as far as
            # possible; remaining columns stay at the last finite values
            V[:, t + 1:] = np.nan_to_num(v, posinf=np.finfo(np.float32).max,
                                         neginf=np.finfo(np.float32).min)[:, None]
            break
    return V.astype(np.float32)


def kernel(y, u, W):
    global LAST_RESULTS
    from concourse.bass_utils import run_bass_kernel_spmd

    y = np.ascontiguousarray(np.asarray(y, dtype=np.float32))
    Vf = _v_table(W)

    colmax = np.abs(Vf).max(axis=0)
    nz = np.nonzero(colmax)[0]
    t_last = int(nz[-1]) + 1 if len(nz) else 0      # preds beyond are exact 0
    n_cols = min(OUT_COLS, (AR + t_last + 9 + 7) & ~7)

    V_full = np.zeros((AR, n_cols), dtype=np.float32)
    V_full[:, :AR] = np.eye(AR, dtype=np.float32)
    V_full[:, AR:] = Vf[:, :n_cols - AR]

    if n_cols not in _nc_cache:
        _nc_cache[n_cols] = _build_nc(n_cols)
    nc = _nc_cache[n_cols]

    in_maps = [
        {"yT": np.ascontiguousarray(y[i * ROWS:(i + 1) * ROWS].T), "V": V_full}
        for i in range(N_CORES)
    ]
    LAST_RESULTS = run_bass_kernel_spmd(nc, in_maps, list(range(N_CORES)))

    out = np.zeros((BATCH, OUT_COLS), dtype=np.float32)
    for i in range(N_CORES):
        out[i * ROWS:(i + 1) * ROWS, :n_cols] = LAST_RESULTS.results[i]["out"]
    return out


# revision 2
# speedup vs baseline: 1.1327x; 1.1327x over previous
"""ARX forward kernel for Trainium2 (8 NeuronCores, data-parallel).

The reference zeroes the exogenous term, so the model is a pure linear
recurrence out[:, t] = sum_k w_k * out[:, t-8+k] with out[:, :8] = y.
Writing the 8x8 companion matrix M (carry_{t+1} = carry_t @ M) gives
pred_t = y @ (M^t w), so the whole 4096-step scan collapses into one
matmul out = y @ [I_8 | V] with V[:, t] = M^t w precomputed on host
(4096 tiny 8-vector iterations, float64).

The recurrence is stable (spectral radius ~0.77 for the 0.05-scaled
weights), so M^t w underflows float32 to exactly 0 after a few hundred
steps; both the reference scan and this kernel produce exact zeros
there.  The device therefore computes and writes only the nonzero
column prefix (determined from V at runtime) and the host pads the
remaining all-zero columns.

Sharding: pure data parallel, batch 8192 -> 1024 rows per core, W/V
replicated, per-core output gathered on host by concatenation.

Device kernel (raw bass, per core): the contraction dim is only 8, so
matmuls are packed 4x via TensorE row tiling (32x128 mode): row groups
at partitions 0/32/64/96 each hold one 128-row batch chunk's yT and a
replica of V, and 4 matmuls run concurrently in the array.  2 rounds
cover the 8 chunks; 8 PSUM banks hold the results, which DVE/ACT
copy to SBUF and HWDGE DMAs stream to DRAM.
"""

import os

import numpy as np

AR = 8
SEQ = 4096
BATCH = 8192
OUT_COLS = SEQ + AR          # 4104
N_CORES = 8
ROWS = BATCH // N_CORES      # 1024
P = 128                      # SBUF/PSUM partitions
MM_CHUNK = 512               # max fp32 matmul free dim / one PSUM bank
N_CHUNKS = ROWS // P         # 8 row chunks per core
N_GRP = 4                    # TensorE row groups (32-row tiling)
N_ROUNDS = N_CHUNKS // N_GRP

_nc_cache = {}
LAST_RESULTS = None          # BassKernelResults of the most recent run


def _build_nc_raw(n_cols):
    """Raw-bass program: out[1024, n_cols] = y_shard @ [I|V] (per core).

    Input layout (host-packed, see _pack_input): one [104, 2*P + n_cols]
    f32 tensor; partitions 32g..32g+7 hold, for row group g:
      cols [r*P, (r+1)*P): yT of batch chunk c = 4r+g   (rounds r=0,1)
      cols [2*P, 2*P+n_cols): V replica
    """
    import concourse.bass as bass
    import concourse.mybir as mybir

    assert n_cols <= MM_CHUNK, "raw kernel assumes single-column-chunk output"
    f32 = mybir.dt.float32
    in_cols = N_ROUNDS * P + n_cols
    v_off = N_ROUNDS * P

    nc = bass.Bass("TRN2", target_bir_lowering=False, debug=False,
                   num_devices=N_CORES)
    inp = nc.dram_tensor("inp", [3 * 32 + AR, in_cols], f32,
                         kind="ExternalInput").ap()
    out = nc.dram_tensor("out", [ROWS, n_cols], f32,
                         kind="ExternalOutput").ap()

    with (
        nc.sbuf_tensor([3 * 32 + AR, in_cols], f32) as inp_t,
        nc.sbuf_tensor([P, N_CHUNKS * n_cols], f32) as out_t,
        nc.psum_tensor([P, N_CHUNKS, MM_CHUNK], f32) as psum_t,
        nc.semaphore() as in_sem,
        nc.semaphore() as mm_sem,
        nc.semaphore() as cpv_sem,
        nc.semaphore() as cps_sem,
        nc.semaphore() as do_sem,
        nc.Block() as block,
    ):
        @block.sync
        def _(sync):
            sync.dma_start(out=inp_t[:], in_=inp).then_inc(in_sem, 16)
            for c in range(N_CHUNKS):
                sem, n = (cpv_sem, c // 2) if c % 2 == 0 else (cps_sem, c // 2)
                sync.wait_ge(sem, n + 1)
                sync.dma_start(
                    out=out[c * P:(c + 1) * P, :],
                    in_=out_t[:, c * n_cols:(c + 1) * n_cols],
                ).then_inc(do_sem, 16)
            sync.wait_ge(do_sem, N_CHUNKS * 16)

        @block.tensor
        def _(tensor):
            tensor.wait_ge(in_sem, 16)
            for r in range(N_ROUNDS):
                for g in range(N_GRP):
                    c = N_GRP * r + g
                    tensor.matmul(
                        psum_t[:, c, :n_cols],
                        inp_t[32 * g:32 * g + AR, r * P:(r + 1) * P],
                        inp_t[32 * g:32 * g + AR, v_off:v_off + n_cols],
                        start=True, stop=True,
                        tile_position=(32 * g, 0),
                    ).then_inc(mm_sem, 1)

        @block.vector
        def _(vector):
            for i in range(N_CHUNKS // 2):
                c = 2 * i
                vector.wait_ge(mm_sem, c + 1)
                vector.tensor_copy(
                    out_t[:, c * n_cols:(c + 1) * n_cols],
                    psum_t[:, c, :n_cols],
                ).then_inc(cpv_sem, 1)

        @block.scalar
        def _(scalar):
            for i in range(N_CHUNKS // 2):
                c = 2 * i + 1
                scalar.wait_ge(mm_sem, c + 1)
                scalar.copy(
                    out_t[:, c * n_cols:(c + 1) * n_cols],
                    psum_t[:, c, :n_cols],
                ).then_inc(cps_sem, 1)

    return nc


def _build_nc_tile(n_cols):
    """Tile-framework fallback (any n_cols)."""
    import concourse.mybir as mybir
    import concourse.tile as tile
    from concourse import bacc

    f32 = mybir.dt.float32
    nc = bacc.Bacc("TRN2", target_bir_lowering=False, debug=False,
                   num_devices=N_CORES)
    yT = nc.dram_tensor("yT", [AR, ROWS], f32, kind="ExternalInput").ap()
    V = nc.dram_tensor("V", [AR, n_cols], f32, kind="ExternalInput").ap()
    out = nc.dram_tensor("out", [ROWS, n_cols], f32,
                         kind="ExternalOutput").ap()

    chunks = [(c, min(MM_CHUNK, n_cols - c)) for c in range(0, n_cols, MM_CHUNK)]

    with tile.TileContext(nc) as tc:
        with (
            tc.tile_pool(name="const", bufs=1) as cpool,
            tc.tile_pool(name="outs", bufs=3) as opool,
            tc.tile_pool(name="psum", bufs=8, space="PSUM") as ppool,
        ):
            yT_t = cpool.tile([AR, ROWS], f32)
            nc.sync.dma_start(yT_t[:], yT)
            V_t = cpool.tile([AR, n_cols], f32)
            nc.sync.dma_start(V_t[:], V)
            for rc in range(ROWS // P):
                ot = opool.tile([P, n_cols], f32, tag="ot")
                for c, wd in chunks:
                    ps = ppool.tile([P, MM_CHUNK], f32, tag="ps")
                    nc.tensor.matmul(
                        ps[:, :wd],
                        yT_t[:, rc * P:(rc + 1) * P],
                        V_t[:, c:c + wd],
                        start=True, stop=True,
                    )
                    nc.vector.tensor_copy(ot[:, c:c + wd], ps[:, :wd])
                nc.sync.dma_start(out[rc * P:(rc + 1) * P, :], ot[:])
    nc.compile()
    return nc


def _v_table(W):
    """V[:, t] = M^t w in float64, cast to float32.  v_{t+1}[0] = w0*v[7],
    v_{t+1}[i] = v[i-1] + w_i*v[7]."""
    w = np.asarray(W, dtype=np.float64)[0, :AR]
    V = np.zeros((AR, SEQ), dtype=np.float64)
    v = w.copy()
    for t in range(SEQ):
        V[:, t] = v
        nv = np.empty(AR)
        nv[0] = 0.0
        nv[1:] = v[:-1]
        nv += w * v[AR - 1]
        v = nv
        if not np.isfinite(v).all():
            # unstable recurrence: remaining columns pinned at f32-max scale
            V[:, t + 1:] = np.nan_to_num(v, posinf=np.finfo(np.float32).max,
                                         neginf=np.finfo(np.float32).min)[:, None]
            break
    return V.astype(np.float32)


def _pack_input(y_shard, V_full):
    """[104, 2*P + n_cols] layout for _build_nc_raw (see its docstring)."""
    n_cols = V_full.shape[1]
    in_cols = N_ROUNDS * P + n_cols
    inp = np.zeros((3 * 32 + AR, in_cols), dtype=np.float32)
    for g in range(N_GRP):
        for r in range(N_ROUNDS):
            c = N_GRP * r + g
            # yT of batch chunk c: [AR, P]
            inp[32 * g:32 * g + AR, r * P:(r + 1) * P] = \
                y_shard[c * P:(c + 1) * P, :].T
        inp[32 * g:32 * g + AR, N_ROUNDS * P:] = V_full
    return inp


def kernel(y, u, W):
    global LAST_RESULTS
    from concourse.bass_utils import run_bass_kernel_spmd

    y = np.ascontiguousarray(np.asarray(y, dtype=np.float32))
    Vf = _v_table(W)

    colmax = np.abs(Vf).max(axis=0)
    nz = np.nonzero(colmax)[0]
    t_last = int(nz[-1]) + 1 if len(nz) else 0      # preds beyond are exact 0
    n_cols = min(OUT_COLS, (AR + t_last + 9 + 7) & ~7)

    V_full = np.zeros((AR, n_cols), dtype=np.float32)
    V_full[:, :AR] = np.eye(AR, dtype=np.float32)
    V_full[:, AR:] = Vf[:, :n_cols - AR]

    impl = os.environ.get("KERNEL_IMPL", "raw")
    if impl == "raw" and n_cols > MM_CHUNK:
        impl = "tile"                               # raw path is prefix-only

    key = (impl, n_cols)
    if key not in _nc_cache:
        _nc_cache[key] = (_build_nc_raw if impl == "raw"
                          else _build_nc_tile)(n_cols)
    nc = _nc_cache[key]

    if impl == "raw":
        in_maps = [
            {"inp": _pack_input(y[i * ROWS:(i + 1) * ROWS], V_full)}
            for i in range(N_CORES)
        ]
    else:
        in_maps = [
            {"yT": np.ascontiguousarray(y[i * ROWS:(i + 1) * ROWS].T),
             "V": V_full}
            for i in range(N_CORES)
        ]
    LAST_RESULTS = run_bass_kernel_spmd(nc, in_maps, list(range(N_CORES)))

    out = np.zeros((BATCH, OUT_COLS), dtype=np.float32)
    for i in range(N_CORES):
        out[i * ROWS:(i + 1) * ROWS, :n_cols] = LAST_RESULTS.results[i]["out"]
    return out


# revision 9
# speedup vs baseline: 1.2022x; 1.0613x over previous
"""ARX forward kernel for Trainium2 (8 NeuronCores, data-parallel).

The reference zeroes the exogenous term, so the model is a pure linear
recurrence out[:, t] = sum_k w_k * out[:, t-8+k] with out[:, :8] = y.
Writing the 8x8 companion matrix M (carry_{t+1} = carry_t @ M) gives
pred_t = y @ (M^t w), so the whole 4096-step scan collapses into one
matmul out = y @ [I_8 | V] with V[:, t] = M^t w precomputed on host
(4096 tiny 8-vector iterations, float64).

The recurrence is stable (spectral radius ~0.77 for the 0.05-scaled
weights), so M^t w underflows float32 to exactly 0 after a few hundred
steps; both the reference scan and this kernel produce exact zeros
there.  The device therefore computes and writes only the nonzero
column prefix (determined from V at runtime) and the host pads the
remaining all-zero columns.

Sharding: pure data parallel, batch 8192 -> 1024 rows per core, W/V
replicated, per-core output gathered on host by concatenation.

Device kernel (raw bass, per core): the contraction dim is only 8, so
matmuls are packed 4x via TensorE row tiling (32x128 mode): row groups
at partitions 0/32/64/96 each hold one 128-row batch chunk's yT and a
replica of V, and 4 matmuls run concurrently in the array.  2 rounds
cover the 8 chunks; 8 PSUM banks hold the results, which DVE/ACT
copy to SBUF and HWDGE DMAs stream to DRAM.
"""

import os

import numpy as np

AR = 8
SEQ = 4096
BATCH = 8192
OUT_COLS = SEQ + AR          # 4104
N_CORES = 8
ROWS = BATCH // N_CORES      # 1024
P = 128                      # SBUF/PSUM partitions
MM_CHUNK = 512               # max fp32 matmul free dim / one PSUM bank
N_CHUNKS = ROWS // P         # 8 row chunks per core
N_GRP = 4                    # TensorE row groups (32-row tiling)
N_ROUNDS = N_CHUNKS // N_GRP

_nc_cache = {}
LAST_RESULTS = None          # BassKernelResults of the most recent run


def _build_nc_raw(n_cols):
    """Raw-bass program: out[1024, n_cols] = y_shard @ [I|V] (per core).

    Input layout (host-packed, see _pack_input): one [104, 2*P + n_cols]
    f32 tensor; partitions 32g..32g+7 hold, for row group g:
      cols [r*P, (r+1)*P): yT of batch chunk c = 4r+g   (rounds r=0,1)
      cols [2*P, 2*P+n_cols): V replica
    """
    import concourse.bass as bass
    import concourse.mybir as mybir

    assert n_cols <= MM_CHUNK, "raw kernel assumes single-column-chunk output"
    f32 = mybir.dt.float32
    in_cols = N_ROUNDS * P + n_cols
    v_off = N_ROUNDS * P

    nc = bass.Bass("TRN2", target_bir_lowering=False, debug=False,
                   num_devices=N_CORES)
    inp = nc.dram_tensor("inp", [3 * 32 + AR, in_cols], f32,
                         kind="ExternalInput").ap()
    out = nc.dram_tensor("out", [ROWS, n_cols], f32,
                         kind="ExternalOutput").ap()

    with (
        nc.sbuf_tensor([3 * 32 + AR, in_cols], f32) as inp_t,
        nc.sbuf_tensor([P, N_CHUNKS * n_cols], f32) as out_t,
        nc.sbuf_tensor([1, 2], f32) as scratch_t,
        nc.psum_tensor([P, N_CHUNKS, MM_CHUNK], f32) as psum_t,
        nc.semaphore() as in_sem,
        nc.semaphore() as mm_sem,
        nc.semaphore() as cpv_sem,
        nc.semaphore() as cps_sem,
        nc.semaphore() as do_sem,
        nc.semaphore() as dummy_sem,
        nc.Block() as block,
    ):
        # input split across the two HWDGE rings (sync + scalar) so the
        # HBM reads overlap; output DMAs likewise alternate rings.
        @block.sync
        def _(sync):
            sync.dma_start(out=inp_t[:, v_off:],
                           in_=inp[:, v_off:]).then_inc(in_sem, 16)
            for i in range(N_CHUNKS // 2):
                c = 2 * i
                sync.wait_ge(cpv_sem, i + 1)
                sync.dma_start(
                    out=out[c * P:(c + 1) * P, :],
                    in_=out_t[:, c * n_cols:(c + 1) * n_cols],
                ).then_inc(do_sem, 16)
            sync.wait_ge(do_sem, N_CHUNKS * 16)

        @block.tensor
        def _(tensor):
            tensor.wait_ge(in_sem, 32)
            for r in range(N_ROUNDS):
                for g in range(N_GRP):
                    c = N_GRP * r + g
                    tensor.matmul(
                        psum_t[:, c, :n_cols],
                        inp_t[32 * g:32 * g + AR, r * P:(r + 1) * P],
                        inp_t[32 * g:32 * g + AR, v_off:v_off + n_cols],
                        start=True, stop=True,
                        tile_position=(32 * g, 0),
                    ).then_inc(mm_sem, 1)

        @block.gpsimd
        def _(gpsimd):
            gpsimd.memset(scratch_t[:, 0:1], 0.0).then_inc(dummy_sem, 1)

        @block.vector
        def _(vector):
            for i in range(N_CHUNKS // 2):
                c = 2 * i
                vector.wait_ge(mm_sem, c + 1)
                vector.tensor_copy(
                    out_t[:, c * n_cols:(c + 1) * n_cols],
                    psum_t[:, c, :n_cols],
                ).then_inc(cpv_sem, 1)

        @block.scalar
        def _(scalar):
            scalar.dma_start(out=inp_t[:, :v_off],
                             in_=inp[:, :v_off]).then_inc(in_sem, 16)
            # dummy op: pull ACT_TABLE_LOAD into the input-DMA wait window
            scalar.wait_ge(dummy_sem, 1)
            scalar.copy(scratch_t[:, 1:2], scratch_t[:, 0:1])
            for i in range(N_CHUNKS // 2):
                c = 2 * i + 1
                scalar.wait_ge(mm_sem, c + 1)
                scalar.copy(
                    out_t[:, c * n_cols:(c + 1) * n_cols],
                    psum_t[:, c, :n_cols],
                ).then_inc(cps_sem, 1)
                # same-engine pipelining: make sure the copy has drained
                # before the DMA reads out_t
                scalar.wait_ge(cps_sem, i + 1)
                scalar.dma_start(
                    out=out[c * P:(c + 1) * P, :],
                    in_=out_t[:, c * n_cols:(c + 1) * n_cols],
                ).then_inc(do_sem, 16)

    return nc


def _build_nc_tile(n_cols):
    """Tile-framework fallback (any n_cols)."""
    import concourse.mybir as mybir
    import concourse.tile as tile
    from concourse import bacc

    f32 = mybir.dt.float32
    nc = bacc.Bacc("TRN2", target_bir_lowering=False, debug=False,
                   num_devices=N_CORES)
    yT = nc.dram_tensor("yT", [AR, ROWS], f32, kind="ExternalInput").ap()
    V = nc.dram_tensor("V", [AR, n_cols], f32, kind="ExternalInput").ap()
    out = nc.dram_tensor("out", [ROWS, n_cols], f32,
                         kind="ExternalOutput").ap()

    chunks = [(c, min(MM_CHUNK, n_cols - c)) for c in range(0, n_cols, MM_CHUNK)]

    with tile.TileContext(nc) as tc:
        with (
            tc.tile_pool(name="const", bufs=1) as cpool,
            tc.tile_pool(name="outs", bufs=3) as opool,
            tc.tile_pool(name="psum", bufs=8, space="PSUM") as ppool,
        ):
            yT_t = cpool.tile([AR, ROWS], f32)
            nc.sync.dma_start(yT_t[:], yT)
            V_t = cpool.tile([AR, n_cols], f32)
            nc.sync.dma_start(V_t[:], V)
            for rc in range(ROWS // P):
                ot = opool.tile([P, n_cols], f32, tag="ot")
                for c, wd in chunks:
                    ps = ppool.tile([P, MM_CHUNK], f32, tag="ps")
                    nc.tensor.matmul(
                        ps[:, :wd],
                        yT_t[:, rc * P:(rc + 1) * P],
                        V_t[:, c:c + wd],
                        start=True, stop=True,
                    )
                    nc.vector.tensor_copy(ot[:, c:c + wd], ps[:, :wd])
                nc.sync.dma_start(out[rc * P:(rc + 1) * P, :], ot[:])
    nc.compile()
    return nc


def _v_table(W):
    """V[:, t] = M^t w in float64, cast to float32.  v_{t+1}[0] = w0*v[7],
    v_{t+1}[i] = v[i-1] + w_i*v[7]."""
    w = np.asarray(W, dtype=np.float64)[0, :AR]
    V = np.zeros((AR, SEQ), dtype=np.float64)
    v = w.copy()
    for t in range(SEQ):
        V[:, t] = v
        nv = np.empty(AR)
        nv[0] = 0.0
        nv[1:] = v[:-1]
        nv += w * v[AR - 1]
        v = nv
        if not np.isfinite(v).all():
            # unstable recurrence: remaining columns pinned at f32-max scale
            V[:, t + 1:] = np.nan_to_num(v, posinf=np.finfo(np.float32).max,
                                         neginf=np.finfo(np.float32).min)[:, None]
            break
    return V.astype(np.float32)


def _pack_input(y_shard, V_full):
    """[104, 2*P + n_cols] layout for _build_nc_raw (see its docstring)."""
    n_cols = V_full.shape[1]
    in_cols = N_ROUNDS * P + n_cols
    inp = np.zeros((3 * 32 + AR, in_cols), dtype=np.float32)
    for g in range(N_GRP):
        for r in range(N_ROUNDS):
            c = N_GRP * r + g
            # yT of batch chunk c: [AR, P]
            inp[32 * g:32 * g + AR, r * P:(r + 1) * P] = \
                y_shard[c * P:(c + 1) * P, :].T
        inp[32 * g:32 * g + AR, N_ROUNDS * P:] = V_full
    return inp


def kernel(y, u, W):
    global LAST_RESULTS
    from concourse.bass_utils import run_bass_kernel_spmd

    y = np.ascontiguousarray(np.asarray(y, dtype=np.float32))
    Vf = _v_table(W)

    colmax = np.abs(Vf).max(axis=0)
    nz = np.nonzero(colmax)[0]
    t_last = int(nz[-1]) + 1 if len(nz) else 0      # preds beyond are exact 0
    n_cols = min(OUT_COLS, (AR + t_last + 9 + 7) & ~7)

    V_full = np.zeros((AR, n_cols), dtype=np.float32)
    V_full[:, :AR] = np.eye(AR, dtype=np.float32)
    V_full[:, AR:] = Vf[:, :n_cols - AR]

    impl = os.environ.get("KERNEL_IMPL", "raw")
    if impl == "raw" and n_cols > MM_CHUNK:
        impl = "tile"                               # raw path is prefix-only

    key = (impl, n_cols)
    if key not in _nc_cache:
        _nc_cache[key] = (_build_nc_raw if impl == "raw"
                          else _build_nc_tile)(n_cols)
    nc = _nc_cache[key]

    if impl == "raw":
        in_maps = [
            {"inp": _pack_input(y[i * ROWS:(i + 1) * ROWS], V_full)}
            for i in range(N_CORES)
        ]
    else:
        in_maps = [
            {"yT": np.ascontiguousarray(y[i * ROWS:(i + 1) * ROWS].T),
             "V": V_full}
            for i in range(N_CORES)
        ]
    LAST_RESULTS = run_bass_kernel_spmd(nc, in_maps, list(range(N_CORES)))

    out = np.zeros((BATCH, OUT_COLS), dtype=np.float32)
    for i in range(N_CORES):
        out[i * ROWS:(i + 1) * ROWS, :n_cols] = LAST_RESULTS.results[i]["out"]
    return out
